# revision 1
# baseline (speedup 1.0000x reference)
"""BitFeedForward (BitNet b1.58-style FFN) on 8 Trainium2 NeuronCores.

Reference computation (per token t, full tensors):
    s_x[t]  = 127/clip(max_d |x[t,d]|, eps);  xq = clip(round(x*s_x),-128,127)/s_x
    s_w     = 1/clip(mean|w|, eps);           wq = clip(round(w*s_w),-1,1)/s_w
    h       = relu(xq @ w1q^T);  (activation-quant h);  y = hq @ w2q^T

Strategy: data-parallel over tokens (1024 tokens/core, weights replicated).
After fake-quant the matmul operands are integers in [-127,127] and ternary
{-1,0,1} - exactly representable in bf16, with fp32-PSUM accumulation exact
(sums < 2^24). So both matmuls run at bf16 peak while the scale divisions are
applied per-token afterwards, matching the f32 reference to ~1e-6.

Per-core schedule:
  S:  per-core shard of sum|w| -> AllReduce([2]) -> means/scales + broadcasts
  X:  per-token absmax -> magic-round quant -> bf16 -> DMA-transpose to [d,t]
  L1: stream w1^T slabs f32 -> JIT-ternarize -> 128x512 accum matmuls -> relu
      -> h (exact ints, f32) to HBM; running elementwise max for per-token max
  M:  cross-partition max via PE transpose; per-token rescale factors
  H:  h -> hq = round(relu_int * g[t]) in bf16, SBUF-resident [128,64,1024]
  L2: stream w2^T slabs -> JIT-ternarize -> accum matmuls -> scale -> out

Weights are transposed on the host (layout prep only; all math on device).
"""
import numpy as np

import concourse.bass as bass
import concourse.mybir as mybir
import concourse.tile as tile
from concourse.bass_utils import run_bass_kernel_spmd
from concourse.masks import make_identity

F32 = mybir.dt.float32
BF16 = mybir.dt.bfloat16
AX = mybir.AxisListType
OP = mybir.AluOpType
ACTF = mybir.ActivationFunctionType

NCORES = 8
MAGIC = 12582912.0          # 1.5 * 2**23: (v + MAGIC) - MAGIC == RNE round(v)
EPS = 1e-5
B, S, D = 4, 2048, 2048     # x: [B, S, D]
O = 8192                    # inner dim
T = (B * S) // NCORES       # tokens per core = 1024
INV_NW = float(np.float32(1.0 / (O * D)))  # 2^-24, exact in f32

KB = D // 128               # 16  k-blocks (layer1 contraction)
NG1 = O // 512              # 16  o-generations (layer1 psum groups)
OB = O // 128               # 64  o-blocks (layer2 contraction)
NG2 = D // 512              # 4   m-generations (layer2 psum groups)
TB = T // 128               # 8   token blocks
TC = T // 512               # 2   token chunks


def _split_excess_waits(nc, max_waits=1):
    """This walrus build rejects >1 sync wait per instruction. Move excess
    waits onto preceding NoOps on the same engine (same-engine program order
    preserves semantics)."""
    for fn in nc.m.functions:
        for blk in fn.blocks:
            out = []
            for inst in blk.instructions:
                si = inst.sync_info
                waits = list(si.on_wait) if si is not None and si.on_wait else []
                if len(waits) > max_waits:
                    extra, keep = waits[:-max_waits], waits[-max_waits:]
                    for i in range(0, len(extra), max_waits):
                        out.append(mybir.InstNoOp(
                            name=f"{inst.name}-wsplit{i}",
                            sync_info=mybir.SyncInfo(
                                on_wait=extra[i:i + max_waits], on_update=[]),
                            bass_nofuse=True,
                            engine=inst.engine,
                        ))
                    si.on_wait = keep
                out.append(inst)
            try:
                blk.instructions = out
            except Exception:
                blk.instructions.clear()
                blk.instructions.extend(out)


def _ternarize(nc, tpf, tpb, w_slab, s_w_bc):
    """f32 slab -> clip(round(w*s_w), -1, 1) in bf16. 1 ACT + 2 DVE ops."""
    shp = list(w_slab.shape)
    t1 = tpf.tile(shp, F32, tag="tern_f32")
    nc.scalar.activation(t1[:], w_slab[:], ACTF.Copy, bias=MAGIC, scale=s_w_bc)
    nc.scalar.activation(t1[:], t1[:], ACTF.Copy, bias=-MAGIC)
    t3 = tpb.tile(shp, BF16, tag="tern_bf")
    nc.vector.tensor_scalar(t3[:], t1[:], 1.0, -1.0, OP.min, OP.max)
    return t3


def build_nc(use_allreduce=True):
    nc = bass.Bass()
    x_in = nc.dram_tensor("x", [T, D], F32, kind="ExternalInput")
    w1t_in = nc.dram_tensor("w1t", [D, O], F32, kind="ExternalInput")
    w2t_in = nc.dram_tensor("w2t", [O, D], F32, kind="ExternalInput")
    wsh1 = nc.dram_tensor("wsh1", [256, 8192], F32, kind="ExternalInput")
    wsh2 = nc.dram_tensor("wsh2", [1024, 2048], F32, kind="ExternalInput")
    y_out = nc.dram_tensor("out", [T, D], F32, kind="ExternalOutput")

    with tile.TileContext(nc) as tc:
        with tc.tile_pool(name="const", bufs=1) as cp, \
             tc.tile_pool(name="dram", bufs=1, space="DRAM") as dram:

            ident = cp.tile([128, 128], F32)
            make_identity(nc, ident[:])
            ones_row = cp.tile([1, 128], F32)
            nc.vector.memset(ones_row[:], 1.0)
            ones_col = cp.tile([128, 1], F32)
            ident_bf = cp.tile([128, 128], BF16)
            nc.vector.memset(ones_col[:], 1.0)
            C = cp.tile([128, TB], F32)      # c[t] = max_x[t] * mu1 / 127
            fscale = cp.tile([128, TB], F32)  # hmaxc[t] * mu2 / 127
            G = cp.tile([128, T], F32)        # g[t] broadcast over partitions

            h_hbm = dram.tile([O, T], F32)

            # ==== L1 era: xqT + macc live, hq not yet ======================
            with tc.tile_pool(name="l1big", bufs=1) as l1p:
                xqT = l1p.tile([128, KB, T], BF16)   # [d-part, k-block, token]
                macc = l1p.tile([128, T], F32)
                nc.vector.memset(macc[:], 0.0)
                xq_dram = dram.tile([T, D], BF16)

                # ---- Phases S+X interleaved: emit the w1-mean chain first,
                # then the x tiles feeding token-chunk 0 and their transposes,
                # then the w2-mean chain and the rest of x, so the first L1
                # matmul's deps (s_w1 + xqT chunk 0) resolve earliest.
                with tc.tile_pool(name="ps_s", bufs=2, space="PSUM") as pss, \
                     tc.tile_pool(name="mu", bufs=2) as mup, \
                     tc.tile_pool(name="xio", bufs=2) as xp, \
                     tc.tile_pool(name="xqp", bufs=5) as xqp, \
                     tc.tile_pool(name="xsc", bufs=2) as xsc:

                    if use_allreduce:
                        chains = [[(wsh1, r * 128, c * 4096, 4096)
                                   for r in range(2) for c in range(2)],
                                  [(wsh2, r * 128, 0, 2048) for r in range(8)]]
                    else:
                        chains = [[(w1t_in, r * 128, c * 4096, 4096)
                                   for r in range(16) for c in range(2)],
                                  [(w2t_in, r * 128, 0, 2048) for r in range(64)]]

                    def emit_mu_chain(j):
                        acc = cp.tile([128, 1], F32, name=f"acc{j}")
                        nc.vector.memset(acc[:], 0.0)
                        for (src, r0, c0, f) in chains[j]:
                            wt = mup.tile([128, 4096], F32, tag="mu")
                            nc.sync.dma_start(wt[:, :f], src[r0:r0 + 128, c0:c0 + f])
                            pr = mup.tile([128, 1], F32, tag="mupart")
                            nc.vector.tensor_reduce(pr[:], wt[:, :f], axis=AX.X,
                                                    op=OP.add,
                                                    apply_absolute_value=True)
                            nc.vector.tensor_tensor(acc[:], acc[:], pr[:], OP.add)
                        pss_t = pss.tile([1, 1], F32, tag="musum", name=f"musum{j}")
                        nc.tensor.matmul(pss_t[:], acc[:], ones_col[:],
                                         start=True, stop=True)
                        summ = cp.tile([1, 1], F32, name=f"sum{j}")
                        if use_allreduce:
                            # AllGather of the 8 scalar partials + local sum:
                            # (N-1) ring steps vs AllReduce's 2(N-1) - halves
                            # the collective latency on the critical path
                            loc = mup.tile([1, 1], F32, tag="loc", name=f"loc{j}")
                            nc.scalar.copy(loc[:], pss_t[:])
                            cc_in = dram.tile([1, 1], F32, name=f"ccin{j}")
                            cc_out = dram.tile([NCORES, 1], F32,
                                               addr_space="Shared",
                                               name=f"ccout{j}")
                            nc.sync.dma_start(cc_in[:], loc[:])
                            nc.gpsimd.collective_compute(
                                "AllGather", OP.bypass,
                                replica_groups=[list(range(NCORES))],
                                ins=[cc_in[:].opt()], outs=[cc_out[:].opt()])
                            srow = mup.tile([1, NCORES], F32, tag="srow",
                                            name=f"srow{j}")
                            nc.sync.dma_start(srow[:],
                                              cc_out[:].rearrange("a b -> b a"))
                            nc.vector.tensor_reduce(summ[:], srow[:], axis=AX.X,
                                                    op=OP.add)
                        else:
                            nc.scalar.copy(summ[:], pss_t[:])
                        # mu_c = max(sum/NW, eps); vals = [1/mu, mu/127]
                        muc = cp.tile([1, 1], F32, name=f"muc{j}")
                        nc.vector.tensor_scalar(muc[:], summ[:], INV_NW, EPS,
                                                OP.mult, OP.max)
                        vals = cp.tile([1, 2], F32, name=f"vals{j}")
                        nc.vector.reciprocal(vals[:, 0:1], muc[:])
                        nc.vector.tensor_scalar_mul(vals[:, 1:2], muc[:],
                                                    1.0 / 127.0)
                        psb = pss.tile([128, 2], F32, tag="bcast", name=f"bc{j}")
                        nc.tensor.matmul(psb[:], ones_row[:], vals[:],
                                         start=True, stop=True)
                        BCj = cp.tile([128, 2], F32, name=f"BC{j}")
                        nc.scalar.copy(BCj[:], psb[:])
                        return BCj

                    def emit_x_tile(xb, mu1_127):
                        xt = xp.tile([128, D], F32, tag="xload")
                        nc.sync.dma_start(xt[:], x_in[xb * 128:(xb + 1) * 128, :])
                        mr = xsc.tile([128, 1], F32, tag="xmax")
                        nc.vector.tensor_reduce(mr[:], xt[:], axis=AX.X,
                                                op=OP.max,
                                                apply_absolute_value=True)
                        mc = xsc.tile([128, 1], F32, tag="xmaxc")
                        nc.vector.tensor_scalar_max(mc[:], mr[:], EPS)
                        nc.vector.tensor_tensor(C[:, xb:xb + 1], mc[:],
                                                mu1_127, OP.mult)
                        rc = xsc.tile([128, 1], F32, tag="xrcp")
                        nc.vector.reciprocal(rc[:], mc[:])
                        sx = xsc.tile([128, 1], F32, tag="xs")
                        nc.vector.tensor_scalar_mul(sx[:], rc[:], 127.0)
                        xr = xp.tile([128, D], F32, tag="xround")
                        nc.scalar.activation(xr[:], xt[:], ACTF.Copy,
                                             bias=MAGIC, scale=sx[:])
                        xq = xqp.tile([128, D], BF16, tag="xq")
                        nc.vector.tensor_scalar_add(xq[:], xr[:], -MAGIC)
                        nc.sync.dma_start(xq_dram[xb * 128:(xb + 1) * 128, :],
                                          xq[:])
                        return xq

                    def emit_transposes(tcc):
                        # [512t, 128d] (DRAM) -> [128d, 512t] (SBUF)
                        for k in range(KB):
                            nc.sync.dma_start_transpose(
                                xqT[:, k, tcc * 512:(tcc + 1) * 512],
                                xq_dram[tcc * 512:(tcc + 1) * 512,
                                        k * 128:(k + 1) * 128])

                    make_identity(nc, ident_bf[:])
                    BC0 = emit_mu_chain(0)
                    mu1_127 = BC0[:, 1:2]
                    xqs0 = [emit_x_tile(xb, mu1_127) for xb in range(4)]
                    # chunk-0 transposes on the otherwise-idle PE so they do
                    # not queue on the DMA engines behind the mu2 shard reads
                    for tb4 in range(4):
                        for k in range(KB):
                            ptt = pss.tile([128, 128], BF16, tag="ptt",
                                           name=f"ptt{tb4}_{k}")
                            nc.tensor.transpose(
                                ptt[:], xqs0[tb4][:, k * 128:(k + 1) * 128],
                                ident_bf[:])
                            nc.scalar.copy(
                                xqT[:, k, tb4 * 128:(tb4 + 1) * 128], ptt[:])
                    BC1 = emit_mu_chain(1)
                    for xb in range(4, TB):
                        emit_x_tile(xb, mu1_127)
                    emit_transposes(1)

                s_w1 = BC0[:, 0:1]
                s_w2 = BC1[:, 0:1]
                mu2_127 = BC1[:, 1:2]

                # ---- Phase L1: h = relu(int matmul), running max ----------
                with tc.tile_pool(name="w1s", bufs=4) as wp, \
                     tc.tile_pool(name="tern1", bufs=3) as tp, \
                     tc.tile_pool(name="tern1b", bufs=18) as tpb, \
                     tc.tile_pool(name="hst", bufs=3) as hst, \
                     tc.tile_pool(name="ps1", bufs=8, space="PSUM") as ps1:
                    for g in range(NG1):
                        pts = [ps1.tile([128, 512], F32, tag="pt", name=f"pt{g}_{i}")
                               for i in range(8)]
                        # pre-ternarize the generation's 16 k-slabs, then run
                        # each psum's k-chain contiguously so psums complete
                        # staggered and their drains overlap the MM stream
                        tslabs = []
                        for k in range(KB):
                            wsl = wp.tile([128, 512], F32, tag="w1slab")
                            nc.sync.dma_start(
                                wsl[:], w1t_in[k * 128:(k + 1) * 128,
                                               g * 512:(g + 1) * 512])
                            tslabs.append(_ternarize(nc, tp, tpb, wsl, s_w1))
                        for ob in range(4):
                            for tc_i in range(TC):
                                for k in range(KB):
                                    nc.tensor.matmul(
                                        pts[ob * TC + tc_i][:],
                                        tslabs[k][:, ob * 128:(ob + 1) * 128],
                                        xqT[:, k, tc_i * 512:(tc_i + 1) * 512],
                                        start=(k == 0), stop=(k == KB - 1))
                        for ob in range(4):
                            hsl = hst.tile([128, T], F32, tag="hslab")
                            for tc_i in range(TC):
                                nc.scalar.activation(
                                    hsl[:, tc_i * 512:(tc_i + 1) * 512],
                                    pts[ob * TC + tc_i][:], ACTF.Relu)
                            nc.vector.tensor_tensor(macc[:], macc[:], hsl[:],
                                                    OP.max)
                            r0 = g * 512 + ob * 128
                            nc.sync.dma_start(h_hbm[r0:r0 + 128, :], hsl[:])

                # ---- Phase M: per-token scales ----------------------------
                with tc.tile_pool(name="ps_m", bufs=2, space="PSUM") as psm, \
                     tc.tile_pool(name="msc", bufs=1) as msc:
                    M1 = msc.tile([128, TB], F32)
                    for tb in range(TB):
                        ptr = psm.tile([128, 128], F32, tag="trp")
                        nc.tensor.transpose(ptr[:],
                                            macc[:, tb * 128:(tb + 1) * 128],
                                            ident[:])
                        nc.vector.tensor_reduce(M1[:, tb:tb + 1], ptr[:],
                                                axis=AX.X, op=OP.max)
                    hmax = msc.tile([128, TB], F32)
                    nc.vector.tensor_tensor(hmax[:], M1[:], C[:], OP.mult)
                    hmaxc = msc.tile([128, TB], F32)
                    nc.vector.tensor_scalar_max(hmaxc[:], hmax[:], EPS)
                    rch = msc.tile([128, TB], F32)
                    nc.vector.reciprocal(rch[:], hmaxc[:])
                    sh = msc.tile([128, TB], F32)
                    nc.vector.tensor_scalar_mul(sh[:], rch[:], 127.0)
                    g_tok = msc.tile([128, TB], F32)
                    nc.vector.tensor_tensor(g_tok[:], C[:], sh[:], OP.mult)
                    nc.vector.tensor_scalar(fscale[:], hmaxc[:], mu2_127, None,
                                            OP.mult)
                    # g_tok [t%128, t//128] -> row [1, T] via PE transpose;
                    # flatten [TB, 128] -> [1, T] through a DRAM bounce (engines
                    # cannot read at unaligned partition offsets)
                    ptg = psm.tile([TB, 128], F32, tag="ptg")
                    nc.tensor.transpose(ptg[:], g_tok[:], ident[:])
                    gsb = msc.tile([TB, 128], F32)
                    nc.scalar.copy(gsb[:], ptg[:])
                    g_dram = dram.tile([TB, 128], F32)
                    nc.sync.dma_start(g_dram[:], gsb[:])
                    g_row = msc.tile([1, T], F32)
                    nc.sync.dma_start(g_row[:],
                                      g_dram[:].rearrange("b t -> (b t)")[None, :])
                    for half in range(T // 512):
                        pg = psm.tile([128, 512], F32, tag="pg")
                        nc.tensor.matmul(pg[:], ones_row[:],
                                         g_row[:, half * 512:(half + 1) * 512],
                                         start=True, stop=True)
                        nc.scalar.copy(G[:, half * 512:(half + 1) * 512], pg[:])

            # ==== L2 era: hq lives =========================================
            # Phase H (hq = round(relu_int * g[t]) in bf16) is fused into the
            # first m-generation of L2 so PE consumes hq[ob] right after it is
            # produced instead of idling while all 64 blocks quantize.
            with tc.tile_pool(name="l2big", bufs=1) as l2p:
                hq = l2p.tile([128, OB, T], BF16)

                with tc.tile_pool(name="hio", bufs=3) as hp, \
                     tc.tile_pool(name="w2s", bufs=4) as wp2, \
                     tc.tile_pool(name="tern2", bufs=4) as tp2, \
                     tc.tile_pool(name="ost", bufs=3) as ostp, \
                     tc.tile_pool(name="ps2", bufs=8, space="PSUM") as ps2:
                    for mg in range(NG2):
                        pts = [ps2.tile([128, 512], F32, tag="pt2", name=f"pt2_{mg}_{i}")
                               for i in range(TB)]
                        for ob in range(OB):
                            if mg == 0:
                                ht = hp.tile([128, T], F32, tag="hload")
                                nc.sync.dma_start(
                                    ht[:], h_hbm[ob * 128:(ob + 1) * 128, :])
                                # in-place h *= G, then round via one fused
                                # 2-op tensor_scalar: (v+M) rounds to f32 (RNE)
                                # between DVE slices, -M restores the integer
                                nc.vector.tensor_tensor(ht[:], ht[:], G[:], OP.mult)
                                nc.vector.tensor_scalar(hq[:, ob, :], ht[:],
                                                        MAGIC, -MAGIC,
                                                        OP.add, OP.add)
                            wsl = wp2.tile([128, 512], F32, tag="w2slab")
                            nc.sync.dma_start(
                                wsl[:], w2t_in[ob * 128:(ob + 1) * 128,
                                               mg * 512:(mg + 1) * 512])
                            t2 = _ternarize(nc, tp2, tp2, wsl, s_w2)
                            for tb in range(TB):
                                nc.tensor.matmul(
                                    pts[tb][:],
                                    hq[:, ob, tb * 128:(tb + 1) * 128],
                                    t2[:], start=(ob == 0), stop=(ob == OB - 1))
                        for tb in range(TB):
                            osb = ostp.tile([128, 512], F32, tag="ostage")
                            nc.scalar.activation(osb[:], pts[tb][:], ACTF.Copy,
                                                 scale=fscale[:, tb:tb + 1])
                            nc.sync.dma_start(
                                y_out[tb * 128:(tb + 1) * 128,
                                      mg * 512:(mg + 1) * 512], osb[:])

    _split_excess_waits(nc)
    return nc


_NC = None


def kernel(x, w1, w2):
    global _NC
    if _NC is None:
        _NC = build_nc()
    x = np.ascontiguousarray(np.asarray(x, dtype=np.float32)).reshape(B * S, D)
    w1t = np.ascontiguousarray(np.asarray(w1, dtype=np.float32).T)  # [D, O]
    w2t = np.ascontiguousarray(np.asarray(w2, dtype=np.float32).T)  # [O, D]
    in_maps = []
    for i in range(NCORES):
        in_maps.append({
            "x": x[i * T:(i + 1) * T],
            "w1t": w1t,
            "w2t": w2t,
            "wsh1": w1t[i * 256:(i + 1) * 256],
            "wsh2": w2t[i * 1024:(i + 1) * 1024],
        })
    res = run_bass_kernel_spmd(_NC, in_maps, core_ids=list(range(NCORES)))
    out = np.concatenate([res.results[i]["out"] for i in range(NCORES)], axis=0)
    return out.reshape(B, S, D)



# revision 14
# speedup vs baseline: 1.0701x; 1.0701x over previous
"""BitFeedForward (BitNet b1.58-style FFN) on 8 Trainium2 NeuronCores.

Reference computation (per token t, full tensors):
    s_x[t]  = 127/clip(max_d |x[t,d]|, eps);  xq = clip(round(x*s_x),-128,127)/s_x
    s_w     = 1/clip(mean|w|, eps);           wq = clip(round(w*s_w),-1,1)/s_w
    h       = relu(xq @ w1q^T);  (activation-quant h);  y = hq @ w2q^T

Strategy: data-parallel over tokens (1024 tokens/core, weights replicated).
After fake-quant the matmul operands are integers in [-127,127] and ternary
{-1,0,1} - exactly representable in bf16, with fp32-PSUM accumulation exact
(sums < 2^24). So both matmuls run at bf16 peak while the scale divisions are
applied per-token afterwards, matching the f32 reference to ~1e-6.

v3 schedule: the w1-|mean| scan (16 small chunks, deep-buffered so the DMA
pipe never waits on reduces), its AllGather, the x quantization and both
token-chunk PE transposes are pipelined so L1 gen0 starts ~47us in (v1:
~125us).  No xq DRAM round-trip: both chunks transpose on the PE through
1-bank psum tiles.  Gen0 runs k-outer so matmuls consume w1 slabs at the
ternarize pipeline rate.  Ternarize is 2 DVE + 1 ACT ops so the ACT queue
never backs up.  The w2-mean chain runs during L1 (scan early / finish late
so its collective never head-of-line-blocks a queue).  Partition
broadcasts/reductions run on GPSIMD instead of PE+PSUM round trips.  h
slabs for o-blocks 60..63 stay in SBUF (skipping their HBM round-trip) and
lead L2's first m-generation; the hq quantization is split mult(Pool 70% /
DVE 30%) + round(DVE) so no engine exceeds the PE rate.  Each L2
m-generation finishes its last 8 o-blocks token-block-major so psum drains
stagger (short epilogue, no psum-reuse stalls at generation boundaries).

Weights are transposed on the host (layout prep only; all math on device).
"""
import numpy as np

import concourse.bass as bass
import concourse.bass_isa as bass_isa
import concourse.mybir as mybir
import concourse.tile as tile
from concourse.bass_utils import run_bass_kernel_spmd
from concourse.masks import make_identity

F32 = mybir.dt.float32
BF16 = mybir.dt.bfloat16
I16 = mybir.dt.int16
AX = mybir.AxisListType
OP = mybir.AluOpType
ACTF = mybir.ActivationFunctionType
RED = bass_isa.ReduceOp

NCORES = 8
MAGIC = 12582912.0          # 1.5 * 2**23: (v + MAGIC) - MAGIC == RNE round(v)
EPS = 1e-5
B, S, D = 4, 2048, 2048     # x: [B, S, D]
O = 8192                    # inner dim
T = (B * S) // NCORES       # tokens per core = 1024
INV_NW = float(np.float32(1.0 / (O * D)))  # 2^-24, exact in f32

KB = D // 128               # 16  k-blocks (layer1 contraction)
NG1 = O // 512              # 16  o-generations (layer1 psum groups)
OB = O // 128               # 64  o-blocks (layer2 contraction)
NG2 = D // 512              # 4   m-generations (layer2 psum groups)
TB = T // 128               # 8   token blocks
TC = T // 512               # 2   token chunks
KEEP0 = 56                  # first o-block whose h slab stays in SBUF
NKEEP = OB - KEEP0          # 4


def _split_excess_waits(nc, max_waits=1):
    """This walrus build rejects >1 sync wait per instruction. Move excess
    waits onto preceding NoOps on the same engine (same-engine program order
    preserves semantics)."""
    for fn in nc.m.functions:
        for blk in fn.blocks:
            out = []
            for inst in blk.instructions:
                si = inst.sync_info
                waits = list(si.on_wait) if si is not None and si.on_wait else []
                if len(waits) > max_waits:
                    extra, keep = waits[:-max_waits], waits[-max_waits:]
                    for i in range(0, len(extra), max_waits):
                        out.append(mybir.InstNoOp(
                            name=f"{inst.name}-wsplit{i}",
                            sync_info=mybir.SyncInfo(
                                on_wait=extra[i:i + max_waits], on_update=[]),
                            bass_nofuse=True,
                            engine=inst.engine,
                        ))
                    si.on_wait = keep
                out.append(inst)
            try:
                blk.instructions = out
            except Exception:
                blk.instructions.clear()
                blk.instructions.extend(out)


def build_nc():
    nc = bass.Bass()
    x_in = nc.dram_tensor("x", [T, D], F32, kind="ExternalInput")
    w1t_in = nc.dram_tensor("w1t", [D, O], F32, kind="ExternalInput")
    w2t_in = nc.dram_tensor("w2t", [O, D], F32, kind="ExternalInput")
    wsh1 = nc.dram_tensor("wsh1", [256, 8192], F32, kind="ExternalInput")
    wsh2 = nc.dram_tensor("wsh2", [1024, 2048], F32, kind="ExternalInput")
    y_out = nc.dram_tensor("out", [T, D], F32, kind="ExternalOutput")

    # 16 chunks of [128,1024] per shard: small enough that a 6-deep buffer
    # keeps the (serializing) DMA pipe ahead of the DVE reduces
    mu_chunks = [[(wsh1, r * 128, c * 1024, 1024)
                  for r in (0, 1) for c in range(8)],
                 [(wsh2, r * 128, c * 1024, 1024)
                  for r in range(8) for c in range(2)]]

    with tile.TileContext(nc) as tc:
        # Pools open/close in strict LIFO (stack) order; long-lived pools
        # (hkeep/hio/w2s/tern2*) therefore open before the L1-era pools.
        cp_cm = tc.tile_pool(name="const", bufs=1)
        cp = cp_cm.__enter__()
        dram_cm = tc.tile_pool(name="dram", bufs=1, space="DRAM")
        dram = dram_cm.__enter__()
        hkp_cm = tc.tile_pool(name="hkeep", bufs=NKEEP)
        hkp = hkp_cm.__enter__()
        hp_cm = tc.tile_pool(name="hio", bufs=5)
        hp = hp_cm.__enter__()
        wp2_cm = tc.tile_pool(name="w2s", bufs=4)
        wp2 = wp2_cm.__enter__()
        tp2_cm = tc.tile_pool(name="tern2", bufs=3)
        tp2 = tp2_cm.__enter__()
        tpb2_cm = tc.tile_pool(name="tern2b", bufs=10)
        tpb2 = tpb2_cm.__enter__()

        ident = cp.tile([128, 128], F32)
        make_identity(nc, ident[:])
        ident_bf = cp.tile([128, 128], BF16)
        make_identity(nc, ident_bf[:])
        C = cp.tile([128, TB], F32)       # c[t] = max_x[t] * mu1 / 127
        fscale = cp.tile([128, TB], F32)  # hmaxc[t] * mu2 / 127
        G = cp.tile([128, T], F32)        # g[t] broadcast over partitions

        h_hbm = dram.tile([KEEP0 * 128, T], I16)

        hkeep_tiles = {}
        for gob in range(KEEP0, OB):
            hkeep_tiles[gob] = hkp.tile([128, T], I16, tag="hkeep",
                                        name=f"hk{gob}")

        # ==== L1-era persistent tiles ======================================
        l1p_cm = tc.tile_pool(name="l1big", bufs=1)
        l1p = l1p_cm.__enter__()
        xqT = l1p.tile([128, KB, T], BF16)   # [d-part, k-block, token]
        macc = l1p.tile([128, T], F32)
        nc.vector.memset(macc[:], 0.0)

        wp_cm = tc.tile_pool(name="w1s", bufs=5)
        wp = wp_cm.__enter__()
        tp_cm = tc.tile_pool(name="tern1", bufs=3)
        tp = tp_cm.__enter__()
        tpb_cm = tc.tile_pool(name="tern1b", bufs=18)
        tpb = tpb_cm.__enter__()
        hst_cm = tc.tile_pool(name="hst", bufs=4)
        hst = hst_cm.__enter__()

        def emit_mu_scan(j, mup):
            """DMA+reduce the |w| shard, partition-sum, fire the AllGather."""
            acc = cp.tile([128, 1], F32, name=f"acc{j}")
            nc.vector.memset(acc[:], 0.0)
            for (src, r0, c0, f) in mu_chunks[j]:
                wt = mup.tile([128, 1024], F32, tag="mu")
                nc.sync.dma_start(wt[:, :f], src[r0:r0 + 128, c0:c0 + f])
                pr = mup.tile([128, 1], F32, tag="mupart")
                nc.vector.tensor_reduce(pr[:], wt[:, :f], axis=AX.X,
                                        op=OP.add, apply_absolute_value=True)
                nc.vector.tensor_tensor(acc[:], acc[:], pr[:], OP.add)
            # gather every core's 128 per-partition partials; the final sum
            # happens on DVE after the collective (no PSUM, no gpsimd math)
            cc_in = dram.tile([128, 1], F32, name=f"ccin{j}")
            cc_out = dram.tile([NCORES * 128, 1], F32, addr_space="Shared",
                               name=f"ccout{j}")
            nc.sync.dma_start(cc_in[:], acc[:])
            nc.gpsimd.collective_compute(
                "AllGather", OP.bypass,
                replica_groups=[list(range(NCORES))],
                ins=[cc_in[:].opt()], outs=[cc_out[:].opt()])
            return cc_out

        def emit_mu_finish(j, mup, cc_out):
            """Reduce the gathered partials to [1/mu, mu/127] (DVE ops)."""
            srow = mup.tile([1, NCORES * 128], F32, tag="srow",
                            name=f"srow{j}")
            nc.scalar.dma_start(srow[:], cc_out[:].rearrange("a b -> b a"))
            summ = cp.tile([1, 1], F32, name=f"sum{j}")
            nc.vector.tensor_reduce(summ[:], srow[:], axis=AX.X, op=OP.add)
            muc = cp.tile([1, 1], F32, name=f"muc{j}")
            nc.vector.tensor_scalar(muc[:], summ[:], INV_NW, EPS,
                                    OP.mult, OP.max)
            vals = cp.tile([1, 2], F32, name=f"vals{j}")
            nc.vector.reciprocal(vals[:, 0:1], muc[:])
            nc.vector.tensor_scalar_mul(vals[:, 1:2], muc[:], 1.0 / 127.0)
            # bounce through DRAM; a stride-0 partition-broadcast DMA fans
            # the two scalars out to all 128 partitions
            vd = dram.tile([1, 2], F32, name=f"vd{j}")
            nc.scalar.dma_start(vd[:], vals[:])
            BCj = cp.tile([128, 2], F32, name=f"BC{j}")
            nc.scalar.dma_start(BCj[:], vd[:].broadcast_to([128, 2]))
            return BCj

        def tern1(slab, s_ptr):
            """f32 slab -> clip(round(w*s_w), -1, 1) bf16 (2 DVE + 1 ACT)."""
            shp = list(slab.shape)
            t1 = tp.tile(shp, F32, tag="tern_f32")
            nc.vector.tensor_scalar(t1[:], slab[:], s_ptr, MAGIC,
                                    OP.mult, OP.add)
            nc.scalar.activation(t1[:], t1[:], ACTF.Copy, bias=-MAGIC)
            t3 = tpb.tile(shp, BF16, tag="tern_bf")
            nc.vector.tensor_scalar(t3[:], t1[:], 1.0, -1.0, OP.min, OP.max)
            return t3

        # ==== Prologue =====================================================
        # ps1a (4 psum banks) outlives the prologue psum pool (ptp), so it
        # opens first; ptp's 4 banks are handed to ps1b after gen0.
        ps1a_cm = tc.tile_pool(name="ps1a", bufs=4, space="PSUM")
        ps1a = ps1a_cm.__enter__()
        mup_cm = tc.tile_pool(name="mu", bufs=3)
        mup = mup_cm.__enter__()
        xp_cm = tc.tile_pool(name="xio", bufs=3)
        xp = xp_cm.__enter__()
        xqp_cm = tc.tile_pool(name="xqp", bufs=6)
        xqp = xqp_cm.__enter__()
        xsc_cm = tc.tile_pool(name="xsc", bufs=2)
        xsc = xsc_cm.__enter__()
        ptp_cm = tc.tile_pool(name="ptp", bufs=4, space="PSUM")
        ptp = ptp_cm.__enter__()

        cc1 = emit_mu_scan(0, mup)

        def emit_x_tile(xb, sub_on_act):
            xt = xp.tile([128, D], F32, tag="xload")
            nc.sync.dma_start(xt[:], x_in[xb * 128:(xb + 1) * 128, :])
            mr = xsc.tile([128, 1], F32, tag="xmax")
            nc.vector.tensor_reduce(mr[:], xt[:], axis=AX.X, op=OP.max,
                                    apply_absolute_value=True)
            mc = xsc.tile([128, 1], F32, tag="xmaxc", name=f"mc{xb}")
            nc.vector.tensor_scalar_max(mc[:], mr[:], EPS)
            rc = xsc.tile([128, 1], F32, tag="xrcp")
            nc.vector.reciprocal(rc[:], mc[:])
            sx = xsc.tile([128, 1], F32, tag="xs")
            nc.vector.tensor_scalar_mul(sx[:], rc[:], 127.0)
            # round in place: xt <- x*sx + MAGIC, then xq = xt - MAGIC (bf16)
            nc.scalar.activation(xt[:], xt[:], ACTF.Copy, bias=MAGIC,
                                 scale=sx[:])
            xq = xqp.tile([128, D], BF16, tag="xq", name=f"xq{xb}")
            if sub_on_act:
                nc.scalar.activation(xq[:], xt[:], ACTF.Copy, bias=-MAGIC)
            else:
                nc.vector.tensor_scalar_add(xq[:], xt[:], -MAGIC)
            return xq, mc

        # x tiles 0-3 (token chunk 0); round+sub on ACT
        xqs = {}
        mcs = {}
        for xb in range(4):
            xq, mc = emit_x_tile(xb, True)
            xqs[xb] = xq
            mcs[xb] = mc

        # w1 gen-0 slabs: DMA right behind x0-3
        g0_slabs = []
        for k in range(KB):
            wsl = wp.tile([128, 512], F32, tag="w1slab", name=f"w1g0k{k}")
            nc.sync.dma_start(wsl[:], w1t_in[k * 128:(k + 1) * 128, 0:512])
            g0_slabs.append(wsl)

        def transpose_chunk(tcc, copy_plan):
            """PE-transpose xq tiles of token chunk tcc into xqT via 1-bank
            [128,512]bf16 psum tiles; copy_plan: list of (k, engine)."""
            for k, eng in copy_plan:
                ptt = ptp.tile([128, 512], BF16, tag="ptt",
                               name=f"ptt{tcc}_{k}")
                for i in range(4):
                    tb4 = tcc * 4 + i
                    nc.tensor.transpose(
                        ptt[:, i * 128:(i + 1) * 128],
                        xqs[tb4][:, k * 128:(k + 1) * 128], ident_bf[:])
                dst = xqT[:, k, tcc * 512:(tcc + 1) * 512]
                if eng == "dve":
                    nc.vector.tensor_copy(dst, ptt[:])
                else:
                    nc.scalar.copy(dst, ptt[:])

        # chunk-0: DVE copies k0-3, then the mu1 finish chain slots into the
        # DVE queue, then DVE k4-7; Pool does par1, k8-11, BC0, k12-15.
        transpose_chunk(0, [(k, "dve") for k in range(4)])
        BC0 = emit_mu_finish(0, mup, cc1)
        transpose_chunk(0, [(k, "dve") for k in range(4, 8)])
        transpose_chunk(0, [(k, "act") for k in range(8, 12)])
        s_w1 = BC0[:, 0:1]
        mu1_127 = BC0[:, 1:2]
        transpose_chunk(0, [(k, "act") for k in range(12, KB)])

        # ternarize gen-0 (needs s_w1); k-interleaved with gen0's matmuls
        g0_tern = [tern1(g0_slabs[k], s_w1) for k in range(KB)]

        # x tiles 4-7 (token chunk 1): round on ACT, sub on DVE
        for xb in range(4, TB):
            xq, mc = emit_x_tile(xb, False)
            xqs[xb] = xq
            mcs[xb] = mc
        for xb in range(TB):
            nc.vector.tensor_tensor(C[:, xb:xb + 1], mcs[xb], mu1_127,
                                    OP.mult)

        # ==== L1 ===========================================================
        # gen0 runs k-outer (tc0 on 4 psums, then tc1) so the PE consumes w1
        # slabs at the ternarize-pipeline rate; chunk-1 transposes slot into
        # the PE queue between the two half-generations.
        hsl0 = {ob: hst.tile([128, T], I16, tag="hslab", name=f"hsl0_{ob}")
                for ob in range(4)}

        def gen0_half(tc_i):
            pts = [ps1a.tile([128, 512], F32, tag="pt",
                             name=f"pt0_{ob}_{tc_i}") for ob in range(4)]
            for k in range(KB):
                for ob in range(4):
                    nc.tensor.matmul(
                        pts[ob][:],
                        g0_tern[k][:, ob * 128:(ob + 1) * 128],
                        xqT[:, k, tc_i * 512:(tc_i + 1) * 512],
                        start=(k == 0), stop=(k == KB - 1))
            for ob in range(4):
                nc.scalar.activation(
                    hsl0[ob][:, tc_i * 512:(tc_i + 1) * 512],
                    pts[ob][:], ACTF.Relu)

        gen0_half(0)
        # chunk-1 transposes (PE) + copies while tc1's deps resolve
        transpose_chunk(1, [(k, "dve") for k in range(8)] +
                           [(k, "act") for k in range(8, KB)])
        gen0_half(1)
        for ob in range(4):
            nc.vector.tensor_tensor(macc[:], macc[:], hsl0[ob][:], OP.max)
            nc.sync.dma_start(h_hbm[ob * 128:(ob + 1) * 128, :], hsl0[ob][:])

        ptp_cm.__exit__(None, None, None)
        xsc_cm.__exit__(None, None, None)
        xqp_cm.__exit__(None, None, None)
        xp_cm.__exit__(None, None, None)
        mup_cm.__exit__(None, None, None)
        ps1b_cm = tc.tile_pool(name="ps1b", bufs=4, space="PSUM")
        ps1b = ps1b_cm.__enter__()

        def emit_gen(g, tslabs):
            pts = [(ps1a if i < 4 else ps1b).tile(
                [128, 512], F32, tag=("pt" if i < 4 else "ptb"),
                name=f"pt{g}_{i}") for i in range(8)]
            for ob in range(4):
                for tc_i in range(TC):
                    for k in range(KB):
                        nc.tensor.matmul(
                            pts[ob * TC + tc_i][:],
                            tslabs[k][:, ob * 128:(ob + 1) * 128],
                            xqT[:, k, tc_i * 512:(tc_i + 1) * 512],
                            start=(k == 0), stop=(k == KB - 1))
            for ob in range(4):
                gob = g * 4 + ob          # global o-block
                if gob >= KEEP0:
                    hsl = hkeep_tiles[gob]
                else:
                    hsl = hst.tile([128, T], I16, tag="hslab")
                for tc_i in range(TC):
                    nc.scalar.activation(
                        hsl[:, tc_i * 512:(tc_i + 1) * 512],
                        pts[ob * TC + tc_i][:], ACTF.Relu)
                nc.vector.tensor_tensor(macc[:], macc[:], hsl[:], OP.max)
                if gob < KEEP0:
                    nc.sync.dma_start(h_hbm[gob * 128:(gob + 1) * 128, :],
                                      hsl[:])

        def load_gen_slabs(g):
            tslabs = []
            for k in range(KB):
                wsl = wp.tile([128, 512], F32, tag="w1slab")
                nc.sync.dma_start(
                    wsl[:], w1t_in[k * 128:(k + 1) * 128,
                                   g * 512:(g + 1) * 512])
                tslabs.append(tern1(wsl, s_w1))
            return tslabs

        emit_gen(1, load_gen_slabs(1))

        # mu2: scan + collective now (runs during gens 2-4); finish after
        # gen 11 so the collective wait never blocks a queue
        mup2_cm = tc.tile_pool(name="mu2", bufs=3)
        mup2 = mup2_cm.__enter__()
        cc2 = emit_mu_scan(1, mup2)

        for g in range(2, 12):
            emit_gen(g, load_gen_slabs(g))

        BC1 = emit_mu_finish(1, mup2, cc2)
        s_w2 = BC1[:, 0:1]
        mu2_127 = BC1[:, 1:2]
        mup2_cm.__exit__(None, None, None)

        def tern2(slab):
            shp = list(slab.shape)
            t1 = tp2.tile(shp, F32, tag="t2_f32")
            nc.vector.tensor_scalar(t1[:], slab[:], s_w2, MAGIC,
                                    OP.mult, OP.add)
            nc.scalar.activation(t1[:], t1[:], ACTF.Copy, bias=-MAGIC)
            t3 = tpb2.tile(shp, BF16, tag="t2_bf")
            nc.vector.tensor_scalar(t3[:], t1[:], 1.0, -1.0, OP.min, OP.max)
            return t3

        for g in range(12, 14):
            emit_gen(g, load_gen_slabs(g))

        # mg0 processes kept o-blocks first, then the streamed ones
        mg0_order = list(range(KEEP0, OB)) + list(range(KEEP0))

        emit_gen(14, load_gen_slabs(14))
        w2_pre = {}
        for ob in mg0_order[:6]:
            wsl = wp2.tile([128, 512], F32, tag="w2slab", name=f"w2pre{ob}")
            nc.sync.dma_start(wsl[:], w2t_in[ob * 128:(ob + 1) * 128, 0:512])
            w2_pre[ob] = tern2(wsl)
        emit_gen(15, load_gen_slabs(15))
        hpre = {}
        for ob in mg0_order[NKEEP:NKEEP + 4]:
            ht = hp.tile([128, T], I16, tag="hload", name=f"hpre{ob}")
            nc.sync.dma_start(ht[:], h_hbm[ob * 128:(ob + 1) * 128, :])
            hpre[ob] = ht

        ps1b_cm.__exit__(None, None, None)
        ps1a_cm.__exit__(None, None, None)
        hst_cm.__exit__(None, None, None)
        tpb_cm.__exit__(None, None, None)
        tp_cm.__exit__(None, None, None)
        wp_cm.__exit__(None, None, None)

        # ==== M: per-token scales =========================================
        with tc.tile_pool(name="ps_m", bufs=2, space="PSUM") as psm, \
             tc.tile_pool(name="msc", bufs=1) as msc:
            M1 = msc.tile([128, TB], F32)
            for tb in range(TB):
                ptr = psm.tile([128, 128], F32, tag="trp")
                nc.tensor.transpose(ptr[:], macc[:, tb * 128:(tb + 1) * 128],
                                    ident[:])
                nc.vector.tensor_reduce(M1[:, tb:tb + 1], ptr[:],
                                        axis=AX.X, op=OP.max)
            hmax = msc.tile([128, TB], F32)
            nc.vector.tensor_tensor(hmax[:], M1[:], C[:], OP.mult)
            hmaxc = msc.tile([128, TB], F32)
            nc.vector.tensor_scalar_max(hmaxc[:], hmax[:], EPS)
            rch = msc.tile([128, TB], F32)
            nc.vector.reciprocal(rch[:], hmaxc[:])
            sh = msc.tile([128, TB], F32)
            nc.vector.tensor_scalar_mul(sh[:], rch[:], 127.0)
            g_tok = msc.tile([128, TB], F32)
            nc.vector.tensor_tensor(g_tok[:], C[:], sh[:], OP.mult)
            nc.vector.tensor_scalar(fscale[:], hmaxc[:], mu2_127, None,
                                    OP.mult)
            # g_tok [t%128, tb] -> [tb, t%128] -> DRAM -> one stride-0
            # partition-broadcast DMA fills G [128, T]
            ptg = psm.tile([TB, 128], F32, tag="ptg")
            nc.tensor.transpose(ptg[:], g_tok[:], ident[:])
            gsb = msc.tile([TB, 128], F32)
            nc.scalar.copy(gsb[:], ptg[:])
            g_dram = dram.tile([TB, 128], F32)
            nc.scalar.dma_start(g_dram[:], gsb[:])
            g_row = g_dram[:].rearrange("b t -> (b t)")[None, :]
            nc.scalar.dma_start(G[:], g_row.broadcast_to([128, T]))

        l1p_cm.__exit__(None, None, None)

        # ==== L2 ===========================================================
        l2p_cm = tc.tile_pool(name="l2big", bufs=1)
        l2p = l2p_cm.__enter__()
        hq = l2p.tile([128, OB, T], BF16)

        def emit_hq(ob, idx):
            """hq[:, ob, :] = round(h * g[t]); h is int16 so the multiply
            must run on DVE; the magic-round splits DVE/Pool 1:7."""
            if ob >= KEEP0:
                ht = hkeep_tiles[ob]
            elif ob in hpre:
                ht = hpre[ob]
            else:
                ht = hp.tile([128, T], I16, tag="hload")
                nc.sync.dma_start(ht[:], h_hbm[ob * 128:(ob + 1) * 128, :])
            hx = hxp.tile([128, T], F32, tag="hx")
            nc.vector.tensor_tensor(hx[:], ht[:], G[:], OP.mult)
            if idx % 8 == 0:
                nc.vector.tensor_scalar(hq[:, ob, :], hx[:], MAGIC, -MAGIC,
                                        OP.add, OP.add)
            else:
                nc.gpsimd.tensor_scalar(hq[:, ob, :], hx[:], MAGIC, -MAGIC,
                                        OP.add, OP.add)

        def get_t2(mg, ob):
            if mg == 0 and ob in w2_pre:
                return w2_pre[ob]
            wsl = wp2.tile([128, 512], F32, tag="w2slab")
            nc.sync.dma_start(wsl[:], w2t_in[ob * 128:(ob + 1) * 128,
                                             mg * 512:(mg + 1) * 512])
            return tern2(wsl)

        with tc.tile_pool(name="hx", bufs=2) as hxp, \
             tc.tile_pool(name="ost", bufs=3) as ostp, \
             tc.tile_pool(name="ps2", bufs=8, space="PSUM") as ps2:
            for mg in range(NG2):
                pts = [ps2.tile([128, 512], F32, tag="pt2",
                                name=f"pt2_{mg}_{i}") for i in range(TB)]
                order = mg0_order if mg == 0 else list(range(OB))
                head, tail = order[:-8], order[-8:]
                for idx, ob in enumerate(head):
                    if mg == 0:
                        emit_hq(ob, idx)
                    t2 = get_t2(mg, ob)
                    for tb in range(TB):
                        nc.tensor.matmul(
                            pts[tb][:], hq[:, ob, tb * 128:(tb + 1) * 128],
                            t2[:], start=(idx == 0), stop=False)
                # tail: tb-major so each psum's chain (and its drain)
                # finishes 8 matmuls apart -> staggered drains
                tail_t2 = {}
                for idx, ob in enumerate(tail):
                    if mg == 0:
                        emit_hq(ob, len(head) + idx)
                    tail_t2[ob] = get_t2(mg, ob)
                for tb in range(TB):
                    for ob in tail:
                        nc.tensor.matmul(
                            pts[tb][:], hq[:, ob, tb * 128:(tb + 1) * 128],
                            tail_t2[ob][:], start=False,
                            stop=(ob == tail[-1]))
                    osb = ostp.tile([128, 512], F32, tag="ostage")
                    if tb % 2 == 0:
                        nc.scalar.activation(osb[:], pts[tb][:], ACTF.Copy,
                                             scale=fscale[:, tb:tb + 1])
                    else:
                        nc.vector.tensor_scalar(osb[:], pts[tb][:],
                                                fscale[:, tb:tb + 1], None,
                                                OP.mult)
                    nc.sync.dma_start(
                        y_out[tb * 128:(tb + 1) * 128,
                              mg * 512:(mg + 1) * 512], osb[:])

        l2p_cm.__exit__(None, None, None)
        tpb2_cm.__exit__(None, None, None)
        tp2_cm.__exit__(None, None, None)
        wp2_cm.__exit__(None, None, None)
        hp_cm.__exit__(None, None, None)
        hkp_cm.__exit__(None, None, None)
        dram_cm.__exit__(None, None, None)
        cp_cm.__exit__(None, None, None)

    _split_excess_waits(nc)
    return nc


_NC = None


def kernel(x, w1, w2):
    global _NC
    if _NC is None:
        _NC = build_nc()
    x = np.ascontiguousarray(np.asarray(x, dtype=np.float32)).reshape(B * S, D)
    w1t = np.ascontiguousarray(np.asarray(w1, dtype=np.float32).T)  # [D, O]
    w2t = np.ascontiguousarray(np.asarray(w2, dtype=np.float32).T)  # [O, D]
    in_maps = []
    for i in range(NCORES):
        in_maps.append({
            "x": x[i * T:(i + 1) * T],
            "w1t": w1t,
            "w2t": w2t,
            "wsh1": w1t[i * 256:(i + 1) * 256],
            "wsh2": w2t[i * 1024:(i + 1) * 1024],
        })
    res = run_bass_kernel_spmd(_NC, in_maps, core_ids=list(range(NCORES)))
    out = np.concatenate([res.results[i]["out"] for i in range(NCORES)], axis=0)
    return out.reshape(B, S, D)
